# revision 14
# baseline (speedup 1.0000x reference)
"""Enformer multi-head attention (relative position) Trainium2 Bass kernel.

Problem: nn_Enformer_20753281974759
  B=2, L=1536, D_MODEL=1536, H=8, DQK=64, DV=192, POS_FEATS=192

Sharding (8 NeuronCores): data-parallel over batch x tensor-parallel over
heads.  Core c handles batch b = c // 4 and heads (2*(c%4), 2*(c%4)+1).
Each core computes a full-width [L, D_MODEL] partial of the output
projection (its 2 heads' contribution); the host sums the 4 partials per
batch and adds bo.

Per-core device pipeline (all fp32):
  phase 1:  Q^T/K^T = W^T x^T (transposed layouts, head-dim on partitions),
            V = x Wv (natural layout, + a per-head ones column)
  phase 1b: rK^T = Wrk^T pe^T (pe is a host-computed constant)
  phase 2:  per (head, 128-query tile):
            rel-window matmul -> diagonal-shift DMA (relative_shift)
            content matmul -> fused add+row-max (tensor_tensor_reduce)
            exp (ACT, bias=-rowmax) -> PE transpose -> attn matmul whose
            ones column yields the softmax denominator for free; the
            normalization folds into the PSUM-evict scale.
  phase 3:  out_partial = attn Wo_c via the attn^T tiles kept from phase 2
"""

import math
from contextlib import ExitStack

import numpy as np

import concourse.bacc as bacc
import concourse.mybir as mybir
import concourse.tile as tile
import bass_rust
from concourse import bass_utils
from concourse.masks import make_identity

# ---------------------------------------------------------------- constants
B, L, DM = 2, 1536, 1536
H, DQK, DV = 8, 64, 192
PF = 192                  # POS_FEATS
R = 2 * L - 1             # 3071 relative positions
P = 128
KO = DM // P              # 12 contraction chunks
NQ = L // P               # 12 query tiles
HL = 2                    # heads per core
DH = HL * DQK             # 128: stacked head dim on partitions
DVL = HL * DV             # 384: local value dim
N_CORES = 8
SCALE = DQK ** -0.5
WREL = L + P - 1          # 1663: rel window width per query tile
WRELP = WREL + 1          # padded to even for f32r matmuls
NEG_INF = -3.0e38

_F32 = mybir.dt.float32
_F32R = mybir.dt.float32r

# row permutation of the per-core Wo slice matching ATT's on-chip layout:
# ATT row (chunk*128 + p) holds global-v ATT_PERM[chunk*128 + p]
ATT_PERM = np.concatenate(
    [
        np.arange(0, 128),        # chunk0: head0 v 0..127
        np.arange(192, 320),      # chunk1: head1 v 0..127
        np.arange(320, 384),      # chunk2[0:64]: head1 v 128..191
        np.arange(128, 192),      # chunk2[64:128]: head0 v 128..191
    ]
)


# ------------------------------------------------------------ pos features
def _pos_features_np():
    """Enformer positional basis [2L-1, PF], matched to the jax-on-CPU
    reference (same float32 op sequence on the CPU backend); numpy
    fallback if jax is unavailable."""
    try:
        import jax
        import jax.numpy as jnp

        cpu = jax.devices("cpu")[0]
        with jax.default_device(cpu):
            pos = jnp.arange(-L + 1, L, dtype=jnp.float32)
            ap = jnp.abs(pos)[:, None]
            nb = PF // 6
            Lf = float(L)
            max_range = np.log(Lf) / np.log(2.0)
            half_life = jnp.asarray(
                2.0 ** np.linspace(3.0, max_range, nb), jnp.float32
            )
            f_exp = jnp.exp(-np.log(2.0) / half_life * ap)
            cw = jnp.asarray(2.0 ** np.arange(1, nb + 1) - 1.0, jnp.float32)
            f_cm = (cw[None, :] > ap).astype(jnp.float32)
            stddev = Lf / (2 * nb)
            mean = jnp.asarray(np.linspace(Lf / nb, Lf, nb), jnp.float32)
            conc = (mean / stddev) ** 2
            rate = mean / stddev**2
            log_unnorm = jnp.where(
                ap > 0, (conc - 1.0) * jnp.log(jnp.maximum(ap, 1e-20)), 0.0
            ) - rate * ap
            log_norm = jax.lax.lgamma(conc) - conc * jnp.log(rate)
            p = jnp.exp(log_unnorm - log_norm) + 1e-8
            f_g = p / jnp.max(p)
            emb = jnp.concatenate([f_exp, f_cm, f_g], axis=-1)
            out = jnp.concatenate([emb, jnp.sign(pos)[:, None] * emb], axis=-1)
            return np.asarray(out, dtype=np.float32)
    except Exception:
        ft = np.float32
        pos = np.arange(-L + 1, L, dtype=ft)
        ap = np.abs(pos)[:, None]
        nb = PF // 6
        Lf = float(L)
        max_range = np.log(Lf) / np.log(2.0)
        half_life = (2.0 ** np.linspace(3.0, max_range, nb)).astype(ft)
        f_exp = np.exp((-ft(np.log(2.0)) / half_life) * ap)
        cw = (2.0 ** np.arange(1, nb + 1) - 1.0).astype(ft)
        f_cm = (cw[None, :] > ap).astype(ft)
        stddev = Lf / (2 * nb)
        mean = np.linspace(Lf / nb, Lf, nb)
        conc = (np.asarray(mean, ft) / ft(stddev)) ** 2
        rate = np.asarray(mean, ft) / ft(stddev) ** 2
        lg = np.array([math.lgamma(float(c)) for c in conc], dtype=ft)
        log_unnorm = np.where(
            ap > 0, (conc - ft(1.0)) * np.log(np.maximum(ap, ft(1e-20))), ft(0.0)
        ) - rate * ap
        log_norm = lg - conc * np.log(rate)
        p = np.exp(log_unnorm - log_norm) + ft(1e-8)
        f_g = p / p.max()
        emb = np.concatenate([f_exp, f_cm, f_g], axis=-1)
        return np.concatenate(
            [emb, np.sign(pos)[:, None] * emb], axis=-1
        ).astype(np.float32)


# ------------------------------------------------------------- bass builder
def _diag_ap(ap, width, out_cols, shift0):
    """AP reading src[p, shift0 - p + j] (j in [0, out_cols)) from a 2-D
    [P, width] SBUF tile: flat element step width-1 walks one partition
    down while stepping one element back."""
    c = ap.copy()
    c.ap = bass_rust.VecI64Pair([[width - 1, P], [1, out_cols]])
    c.offset = ap.offset + shift0
    return c


def build_kernel(loop_T=None):
    nc = bacc.Bacc("TRN2", target_bir_lowering=False, debug=False,
                   num_devices=N_CORES)

    xT = nc.dram_tensor("xT", [DM, L], _F32R, kind="ExternalInput")
    wq = nc.dram_tensor("wq", [DM, DH], _F32R, kind="ExternalInput")
    wk = nc.dram_tensor("wk", [DM, DH], _F32R, kind="ExternalInput")
    wv = nc.dram_tensor("wv", [DM, DVL], _F32R, kind="ExternalInput")
    wrk = nc.dram_tensor("wrk", [PF, DH], _F32R, kind="ExternalInput")
    peT = nc.dram_tensor("peT", [PF, R], _F32R, kind="ExternalInput")
    bw = nc.dram_tensor("bw", [DH, 1], _F32, kind="ExternalInput")
    br = nc.dram_tensor("br", [DH, 1], _F32, kind="ExternalInput")
    wo = nc.dram_tensor("wo", [DVL, DM], _F32R, kind="ExternalInput")
    y = nc.dram_tensor("y", [L, DM], _F32, kind="ExternalOutput")

    with tile.TileContext(nc) as tc, ExitStack() as ctx:
        persist = ctx.enter_context(tc.tile_pool(name="persist", bufs=1))

        QwT = persist.tile([P, L], _F32R)       # Q^T*scale + r_w_bias
        QrT = persist.tile([P, L], _F32R)       # Q^T*scale + r_r_bias
        KT = persist.tile([P, L], _F32R)
        rKT = persist.tile([P, R + 1], _F32R)
        Vg = persist.tile([P, KO, HL, DV + 2], _F32R)  # per-head V + ones col
        ATT = persist.tile([P, 3, L], _F32R)    # attn^T (normalized)
        bwT = persist.tile([P, 1], _F32)
        brT = persist.tile([P, 1], _F32)
        ident = persist.tile([P, P], _F32)

        nc.sync.dma_start(bwT[:], bw[:])
        nc.sync.dma_start(brT[:], br[:])
        make_identity(nc, ident[:])
        ones32 = persist.tile([P, KO, HL, 2], _F32)
        nc.vector.memset(ones32[:, :, :, 0], 1.0)
        nc.vector.memset(ones32[:, :, :, 1], 0.0)
        nc.any.tensor_copy(Vg[:, :, :, DV : DV + 2], ones32[:])

        import contextlib
        loop_cm = tc.For_i(0, loop_T, 1) if loop_T else contextlib.nullcontext()
        ctx.enter_context(loop_cm)

        # ---------------- phase 1: projections ----------------
        with ExitStack() as p1:
            xpool = p1.enter_context(tc.tile_pool(name="xTp", bufs=1))
            wpool = p1.enter_context(tc.tile_pool(name="wproj", bufs=1))
            qk_ps = p1.enter_context(
                tc.tile_pool(name="qk_ps", bufs=1, space="PSUM")
            )

            xT_sb = xpool.tile([P, KO, L], _F32R)
            xTr = xT[:].rearrange("(ko p) i -> p ko i", p=P)
            for mo in range(KO):
                nc.sync.dma_start(xT_sb[:, mo, :], xTr[:, mo, :])

            wq_sb = wpool.tile([P, KO, DH], _F32R)
            wk_sb = wpool.tile([P, KO, DH], _F32R)
            wv_sb = wpool.tile([P, KO, DVL], _F32R)
            nc.sync.dma_start(wq_sb[:], wq[:].rearrange("(ko p) d -> p ko d", p=P))
            nc.sync.dma_start(wk_sb[:], wk[:].rearrange("(ko p) d -> p ko d", p=P))
            nc.sync.dma_start(wv_sb[:], wv[:].rearrange("(ko p) d -> p ko d", p=P))

            # Q^T and K^T: [DH=128, L], accumulated over 12 m-chunks
            qt_ps = [qk_ps.tile([P, 512], _F32, tag=f"qt{c}", name=f"qt{c}") for c in range(3)]
            kt_ps = [qk_ps.tile([P, 512], _F32, tag=f"kt{c}", name=f"kt{c}") for c in range(3)]
            for mo in range(KO):
                for c in range(3):
                    sl = slice(c * 512, (c + 1) * 512)
                    nc.tensor.matmul(
                        qt_ps[c][:], wq_sb[:, mo, :], xT_sb[:, mo, sl],
                        start=(mo == 0), stop=(mo == KO - 1),
                    )
                    nc.tensor.matmul(
                        kt_ps[c][:], wk_sb[:, mo, :], xT_sb[:, mo, sl],
                        start=(mo == 0), stop=(mo == KO - 1),
                    )
            for c in range(3):
                sl = slice(c * 512, (c + 1) * 512)
                nc.scalar.activation(
                    QwT[:, sl], qt_ps[c][:],
                    mybir.ActivationFunctionType.Identity,
                    bias=bwT[:], scale=SCALE,
                )
                nc.scalar.activation(
                    QrT[:, sl], qt_ps[c][:],
                    mybir.ActivationFunctionType.Identity,
                    bias=brT[:], scale=SCALE,
                )
                nc.any.tensor_copy(KT[:, sl], kt_ps[c][:])

            # V: [L, DVL] natural layout, per 128-row tile
            with tc.tile_pool(name="v_ps", bufs=2, space="PSUM") as v_ps_pool:
                for jo in range(KO):
                    v_ps = v_ps_pool.tile([P, DVL], _F32)
                    for mo in range(KO):
                        nc.tensor.matmul(
                            v_ps[:],
                            xT_sb[:, mo, jo * P : (jo + 1) * P],
                            wv_sb[:, mo, :],
                            start=(mo == 0), stop=(mo == KO - 1),
                        )
                    for h in range(HL):
                        nc.any.tensor_copy(
                            Vg[:, jo, h, :DV], v_ps[:, h * DV : (h + 1) * DV]
                        )

        # ---------------- phase 1b: rKT = Wrk^T @ peT ----------------
        with ExitStack() as p15:
            pepool = p15.enter_context(tc.tile_pool(name="pe", bufs=1))
            rps = p15.enter_context(tc.tile_pool(name="r_ps", bufs=2, space="PSUM"))
            pe0 = pepool.tile([P, R + 1], _F32R)
            pe1 = pepool.tile([PF - P, R + 1], _F32R)
            wrk0 = pepool.tile([P, DH], _F32R)
            wrk1 = pepool.tile([PF - P, DH], _F32R)
            nc.sync.dma_start(pe0[:, :R], peT[:P, :])
            nc.sync.dma_start(pe1[:, :R], peT[P:, :])
            zcol = pepool.tile([P, 1], _F32)
            nc.vector.memset(zcol[:], 0.0)
            nc.any.tensor_copy(pe0[:, R : R + 1], zcol[:])
            nc.any.tensor_copy(pe1[:, R : R + 1], zcol[: PF - P])
            nc.sync.dma_start(wrk0[:], wrk[:P, :])
            nc.sync.dma_start(wrk1[:], wrk[P:, :])
            for rc in range(6):
                lo = rc * 512
                ps = rps.tile([P, 512], _F32)
                nc.tensor.matmul(
                    ps[:], wrk0[:], pe0[:, lo : lo + 512], start=True, stop=False
                )
                nc.tensor.matmul(
                    ps[:], wrk1[:], pe1[:, lo : lo + 512], start=False, stop=True
                )
                nc.any.tensor_copy(rKT[:, lo : lo + 512], ps[:])

        # ---------------- phase 2: attention ----------------
        with ExitStack() as p2:
            relwin_pool = p2.enter_context(tc.tile_pool(name="relwin", bufs=2))
            relsh_pool = p2.enter_context(tc.tile_pool(name="relsh", bufs=2))
            s_pool = p2.enter_context(tc.tile_pool(name="sS", bufs=2))
            w_pool = p2.enter_context(tc.tile_pool(name="wS", bufs=2))
            wt_pool = p2.enter_context(tc.tile_pool(name="wt", bufs=2))
            small_pool = p2.enter_context(tc.tile_pool(name="small", bufs=4))
            rel_ps = p2.enter_context(tc.tile_pool(name="rel_ps", bufs=2, space="PSUM"))
            s_ps = p2.enter_context(tc.tile_pool(name="s_ps", bufs=2, space="PSUM"))
            t_ps = p2.enter_context(tc.tile_pool(name="t_ps", bufs=2, space="PSUM"))
            a_ps = p2.enter_context(tc.tile_pool(name="a_ps", bufs=2, space="PSUM"))

            for h in range(HL):
                hsl = slice(h * DQK, (h + 1) * DQK)
                for iq in range(NQ):
                    i0 = iq * P
                    isl = slice(i0, i0 + P)
                    c0 = (L - P) - i0  # rel-window start in rel coords

                    # rel window [P, WREL] (padded width, col WREL unused)
                    relwin = relwin_pool.tile([P, WRELP], _F32)
                    for rc in range(4):
                        lo = rc * 512
                        w = min(512, WRELP - lo)
                        ps = rel_ps.tile([P, 512], _F32)
                        nc.tensor.matmul(
                            ps[:, :w], QrT[hsl, isl],
                            rKT[hsl, c0 + lo : c0 + lo + w],
                            start=True, stop=True,
                        )
                        nc.any.tensor_copy(relwin[:, lo : lo + w], ps[:, :w])

                    # diagonal shift: relsh[p, j] = relwin[p, (P-1) - p + j]
                    relsh = relsh_pool.tile([P, L], _F32)
                    nc.sync.dma_start(relsh[:], _diag_ap(relwin[:], WRELP, L, P - 1))

                    # content + relsh, then row-max
                    S = s_pool.tile([P, L], _F32)
                    for jc in range(3):
                        sl = slice(jc * 512, (jc + 1) * 512)
                        ps = s_ps.tile([P, 512], _F32)
                        nc.tensor.matmul(
                            ps[:], QwT[hsl, isl], KT[hsl, sl],
                            start=True, stop=True,
                        )
                        nc.vector.tensor_tensor(
                            S[:, sl], ps[:], relsh[:, sl], mybir.AluOpType.add
                        )
                    m = small_pool.tile([P, 1], _F32, tag="m")
                    nc.vector.reduce_max(m[:], S[:], axis=mybir.AxisListType.X)
                    negm = small_pool.tile([P, 1], _F32, tag="negm")
                    nc.vector.tensor_scalar_mul(negm[:], m[:], -1.0)

                    # exp(S - rowmax)
                    w_sb = w_pool.tile([P, L], _F32)
                    nc.scalar.activation(
                        w_sb[:], S[:], mybir.ActivationFunctionType.Exp,
                        bias=negm[:], scale=1.0,
                    )

                    # transpose w -> wT blocks [j-part, i]
                    wT = wt_pool.tile([P, NQ, P], _F32R)
                    for g in range(3):
                        tp = t_ps.tile([P, 512], _F32, tag="tp")
                        for k in range(4):
                            jb = g * 4 + k
                            nc.tensor.transpose(
                                tp[:, k * P : (k + 1) * P],
                                w_sb[:, jb * P : (jb + 1) * P],
                                ident[:],
                            )
                        nc.any.tensor_copy(wT[:, g * 4 : (g + 1) * 4, :], tp[:])

                    # attn (+denominator via ones column)
                    aps = a_ps.tile([P, DV + 2], _F32, tag="aps")
                    for jb in range(NQ):
                        nc.tensor.matmul(
                            aps[:], wT[:, jb, :], Vg[:, jb, h, :],
                            start=(jb == 0), stop=(jb == NQ - 1),
                        )
                    rec = small_pool.tile([P, 1], _F32, tag="rec")
                    nc.vector.reciprocal(rec[:], aps[:, DV : DV + 1])
                    anorm = small_pool.tile([P, DV], _F32, tag="anorm")
                    nc.scalar.activation(
                        anorm[:], aps[:, :DV],
                        mybir.ActivationFunctionType.Copy, scale=rec[:],
                    )

                    # transpose attn into ATT. Transpose outputs must land at
                    # PSUM partition 0, so ATT's row order is a host-side
                    # permutation of v (see ATT_PERM; wo rows are permuted to
                    # match):
                    #   chunk0       = h0 local [0:128)
                    #   chunk1       = h1 local [0:128)
                    #   chunk2[0:64] = h1 local [128:192)
                    #   chunk2[64:]  = h0 local [128:192)
                    tpa = t_ps.tile([P, 512], _F32, tag="tp")
                    nc.tensor.transpose(tpa[:, :P], anorm[:, :P], ident[:])
                    nc.any.tensor_copy(ATT[:, h, isl], tpa[:, :P])
                    if h == 0:
                        # rows 64:128 of T(anorm[:, 64:192]) = local v 128:192
                        nc.tensor.transpose(
                            tpa[:, P : 2 * P], anorm[:, 64:192], ident[:]
                        )
                        nc.any.tensor_copy(
                            ATT[64:128, 2, isl], tpa[64:128, P : 2 * P]
                        )
                    else:
                        # T(anorm[:, 128:192]) -> rows 0:64 = local v 128:192
                        nc.tensor.transpose(
                            tpa[:64, P : 2 * P], anorm[:, 128:192], ident[:]
                        )
                        nc.any.tensor_copy(
                            ATT[0:64, 2, isl], tpa[0:64, P : 2 * P]
                        )

        # ---------------- phase 3: output projection ----------------
        with ExitStack() as p3:
            wopool = p3.enter_context(tc.tile_pool(name="wop", bufs=1))
            o_ps = p3.enter_context(tc.tile_pool(name="o_ps", bufs=2, space="PSUM"))
            obuf = p3.enter_context(tc.tile_pool(name="obuf", bufs=3))
            wo_sb = wopool.tile([P, 3, DM], _F32R)
            nc.sync.dma_start(wo_sb[:], wo[:].rearrange("(c p) n -> p c n", p=P))
            for iq in range(NQ):
                isl = slice(iq * P, (iq + 1) * P)
                # c outer / nck inner keeps each ATT block loaded in the PE
                # across 3 matmuls (1 LDWEIGHTS per block instead of 3)
                pss = [
                    o_ps.tile([P, 512], _F32, tag=f"o{nck}", name=f"o{nck}")
                    for nck in range(3)
                ]
                for c in range(3):
                    for nck in range(3):
                        nsl = slice(nck * 512, (nck + 1) * 512)
                        nc.tensor.matmul(
                            pss[nck][:], ATT[:, c, isl], wo_sb[:, c, nsl],
                            start=(c == 0), stop=(c == 2),
                        )
                for nck in range(3):
                    nsl = slice(nck * 512, (nck + 1) * 512)
                    ob = obuf.tile([P, 512], _F32)
                    nc.any.tensor_copy(ob[:], pss[nck][:])
                    nc.sync.dma_start(y[isl, nsl], ob[:])

    nc.compile()
    return nc


# ------------------------------------------------------------------ runner
_CACHE = {}


def _get_nc():
    if "nc" not in _CACHE:
        _CACHE["nc"] = build_kernel()
        _CACHE["peT"] = np.ascontiguousarray(_pos_features_np().T)
    return _CACHE["nc"], _CACHE["peT"]


def make_in_maps(x, Wq, Wk, Wv, Wrk, r_w_bias, r_r_bias, Wo, peT):
    in_maps = []
    for c in range(N_CORES):
        b, hp = divmod(c, 4)
        h0 = 2 * hp
        qsl = slice(h0 * DQK, h0 * DQK + DH)
        vsl = slice(h0 * DV, h0 * DV + DVL)
        in_maps.append(
            {
                "xT": np.ascontiguousarray(x[b].T),
                "wq": np.ascontiguousarray(Wq[:, qsl]),
                "wk": np.ascontiguousarray(Wk[:, qsl]),
                "wv": np.ascontiguousarray(Wv[:, vsl]),
                "wrk": np.ascontiguousarray(Wrk[:, qsl]),
                "peT": peT,
                "bw": np.ascontiguousarray(
                    r_w_bias[0, h0 : h0 + HL, 0, :].reshape(DH, 1)
                ),
                "br": np.ascontiguousarray(
                    r_r_bias[0, h0 : h0 + HL, 0, :].reshape(DH, 1)
                ),
                "wo": np.ascontiguousarray(Wo[vsl, :][ATT_PERM, :]),
            }
        )
    return in_maps


def kernel(x, Wq, Wk, Wv, Wrk, r_w_bias, r_r_bias, Wo, bo, **run_kwargs):
    x = np.asarray(x, np.float32)
    Wq = np.asarray(Wq, np.float32)
    Wk = np.asarray(Wk, np.float32)
    Wv = np.asarray(Wv, np.float32)
    Wrk = np.asarray(Wrk, np.float32)
    r_w_bias = np.asarray(r_w_bias, np.float32)
    r_r_bias = np.asarray(r_r_bias, np.float32)
    Wo = np.asarray(Wo, np.float32)
    bo = np.asarray(bo, np.float32)

    nc, peT = _get_nc()
    in_maps = make_in_maps(x, Wq, Wk, Wv, Wrk, r_w_bias, r_r_bias, Wo, peT)
    res = bass_utils.run_bass_kernel_spmd(
        nc, in_maps, core_ids=list(range(N_CORES)), **run_kwargs
    )
    out = np.zeros((B, L, DM), np.float32)
    for c in range(N_CORES):
        out[c // 4] += res.results[c]["y"]
    out += bo[None, None, :]
    if run_kwargs:
        _CACHE["last_results"] = res
    return out


# revision 21
# speedup vs baseline: 9614.9095x; 9614.9095x over previous
"""Enformer multi-head attention (relative position) Trainium2 Bass kernel.

Problem: nn_Enformer_20753281974759
  B=2, L=1536, D_MODEL=1536, H=8, DQK=64, DV=192, POS_FEATS=192

Sharding (8 NeuronCores): data-parallel over batch x tensor-parallel over
heads.  Core c handles batch b = c // 4 and heads (2*(c%4), 2*(c%4)+1).
Each core computes a full-width [L, D_MODEL] partial of the output
projection (its 2 heads' contribution); the host sums the 4 partials per
batch and adds bo.

Per-core device pipeline (all fp32):
  phase 1:  Q^T/K^T = W^T x^T (transposed layouts, head-dim on partitions),
            V = x Wv (natural layout, + a per-head ones column)
  phase 1b: rK^T = Wrk^T pe^T (pe is a host-computed constant)
  phase 2:  per (head, 128-query tile):
            rel-window matmul -> diagonal-shift DMA (relative_shift)
            content matmul -> fused add+row-max (tensor_tensor_reduce)
            exp (ACT, bias=-rowmax) -> PE transpose -> attn matmul whose
            ones column yields the softmax denominator for free; the
            normalization folds into the PSUM-evict scale.
  phase 3:  out_partial = attn Wo_c via the attn^T tiles kept from phase 2
"""

import math
from contextlib import ExitStack

import numpy as np

import concourse.bacc as bacc
import concourse.mybir as mybir
import concourse.tile as tile
import bass_rust
from concourse import bass_utils
from concourse.masks import make_identity

# ---------------------------------------------------------------- constants
B, L, DM = 2, 1536, 1536
H, DQK, DV = 8, 64, 192
PF = 192                  # POS_FEATS
R = 2 * L - 1             # 3071 relative positions
P = 128
KO = DM // P              # 12 contraction chunks
NQ = L // P               # 12 query tiles
HL = 2                    # heads per core
DH = HL * DQK             # 128: stacked head dim on partitions
DVL = HL * DV             # 384: local value dim
N_CORES = 8
SCALE = DQK ** -0.5
WREL = L + P - 1          # 1663: rel window width per query tile
WRELP = WREL + 1          # padded to even for f32r matmuls
NEG_INF = -3.0e38

_F32 = mybir.dt.float32
_F32R = mybir.dt.float32r

# row permutation of the per-core Wo slice matching ATT's on-chip layout:
# ATT row (chunk*128 + p) holds global-v ATT_PERM[chunk*128 + p]
ATT_PERM = np.concatenate(
    [
        np.arange(0, 128),        # chunk0: head0 v 0..127
        np.arange(192, 320),      # chunk1: head1 v 0..127
        np.arange(320, 384),      # chunk2[0:64]: head1 v 128..191
        np.arange(128, 192),      # chunk2[64:128]: head0 v 128..191
    ]
)


# ------------------------------------------------------------ pos features
def _pos_features_np():
    """Enformer positional basis [2L-1, PF], matched to the jax-on-CPU
    reference (same float32 op sequence on the CPU backend); numpy
    fallback if jax is unavailable."""
    try:
        import jax
        import jax.numpy as jnp

        cpu = jax.devices("cpu")[0]
        with jax.default_device(cpu):
            pos = jnp.arange(-L + 1, L, dtype=jnp.float32)
            ap = jnp.abs(pos)[:, None]
            nb = PF // 6
            Lf = float(L)
            max_range = np.log(Lf) / np.log(2.0)
            half_life = jnp.asarray(
                2.0 ** np.linspace(3.0, max_range, nb), jnp.float32
            )
            f_exp = jnp.exp(-np.log(2.0) / half_life * ap)
            cw = jnp.asarray(2.0 ** np.arange(1, nb + 1) - 1.0, jnp.float32)
            f_cm = (cw[None, :] > ap).astype(jnp.float32)
            stddev = Lf / (2 * nb)
            mean = jnp.asarray(np.linspace(Lf / nb, Lf, nb), jnp.float32)
            conc = (mean / stddev) ** 2
            rate = mean / stddev**2
            log_unnorm = jnp.where(
                ap > 0, (conc - 1.0) * jnp.log(jnp.maximum(ap, 1e-20)), 0.0
            ) - rate * ap
            log_norm = jax.lax.lgamma(conc) - conc * jnp.log(rate)
            p = jnp.exp(log_unnorm - log_norm) + 1e-8
            f_g = p / jnp.max(p)
            emb = jnp.concatenate([f_exp, f_cm, f_g], axis=-1)
            out = jnp.concatenate([emb, jnp.sign(pos)[:, None] * emb], axis=-1)
            return np.asarray(out, dtype=np.float32)
    except Exception:
        ft = np.float32
        pos = np.arange(-L + 1, L, dtype=ft)
        ap = np.abs(pos)[:, None]
        nb = PF // 6
        Lf = float(L)
        max_range = np.log(Lf) / np.log(2.0)
        half_life = (2.0 ** np.linspace(3.0, max_range, nb)).astype(ft)
        f_exp = np.exp((-ft(np.log(2.0)) / half_life) * ap)
        cw = (2.0 ** np.arange(1, nb + 1) - 1.0).astype(ft)
        f_cm = (cw[None, :] > ap).astype(ft)
        stddev = Lf / (2 * nb)
        mean = np.linspace(Lf / nb, Lf, nb)
        conc = (np.asarray(mean, ft) / ft(stddev)) ** 2
        rate = np.asarray(mean, ft) / ft(stddev) ** 2
        lg = np.array([math.lgamma(float(c)) for c in conc], dtype=ft)
        log_unnorm = np.where(
            ap > 0, (conc - ft(1.0)) * np.log(np.maximum(ap, ft(1e-20))), ft(0.0)
        ) - rate * ap
        log_norm = lg - conc * np.log(rate)
        p = np.exp(log_unnorm - log_norm) + ft(1e-8)
        f_g = p / p.max()
        emb = np.concatenate([f_exp, f_cm, f_g], axis=-1)
        return np.concatenate(
            [emb, np.sign(pos)[:, None] * emb], axis=-1
        ).astype(np.float32)


# ------------------------------------------------------------- bass builder
def _diag_ap(ap, width, out_cols, shift0):
    """AP reading src[p, shift0 - p + j] (j in [0, out_cols)) from a 2-D
    [P, width] SBUF tile: flat element step width-1 walks one partition
    down while stepping one element back."""
    c = ap.copy()
    c.ap = bass_rust.VecI64Pair([[width - 1, P], [1, out_cols]])
    c.offset = ap.offset + shift0
    return c


def build_kernel(loop_T=None):
    nc = bacc.Bacc("TRN2", target_bir_lowering=False, debug=False,
                   num_devices=N_CORES)

    xT = nc.dram_tensor("xT", [DM, L], _F32R, kind="ExternalInput")
    wq = nc.dram_tensor("wq", [DM, DH], _F32R, kind="ExternalInput")
    wk = nc.dram_tensor("wk", [DM, DH], _F32R, kind="ExternalInput")
    wv = nc.dram_tensor("wv", [DM, DVL], _F32R, kind="ExternalInput")
    wrk = nc.dram_tensor("wrk", [PF, DH], _F32R, kind="ExternalInput")
    peT = nc.dram_tensor("peT", [PF, R], _F32R, kind="ExternalInput")
    bw = nc.dram_tensor("bw", [DH, 1], _F32, kind="ExternalInput")
    br = nc.dram_tensor("br", [DH, 1], _F32, kind="ExternalInput")
    wo = nc.dram_tensor("wo", [DVL, DM], _F32R, kind="ExternalInput")
    y = nc.dram_tensor("y", [L, DM], _F32, kind="ExternalOutput")

    with tile.TileContext(nc) as tc, ExitStack() as ctx:
        persist = ctx.enter_context(tc.tile_pool(name="persist", bufs=1))

        QwT = persist.tile([P, L], _F32R)       # Q^T*scale + r_w_bias
        QrT = persist.tile([P, L], _F32R)       # Q^T*scale + r_r_bias
        KT = persist.tile([P, L], _F32R)
        rKT = persist.tile([P, R + 1], _F32R)
        Vg = persist.tile([P, KO, HL, DV], _F32R)      # per-head V
        ATT = persist.tile([P, 3, L], _F32R)    # attn^T (normalized)
        bwT = persist.tile([P, 1], _F32)
        brT = persist.tile([P, 1], _F32)
        ident = persist.tile([P, P], _F32)

        nc.sync.dma_start(bwT[:], bw[:])
        nc.sync.dma_start(brT[:], br[:])
        make_identity(nc, ident[:])

        import contextlib
        loop_cm = tc.For_i(0, loop_T, 1) if loop_T else contextlib.nullcontext()
        ctx.enter_context(loop_cm)

        # ---------------- phase 1: projections ----------------
        with ExitStack() as p1:
            xpool = p1.enter_context(tc.tile_pool(name="xTp", bufs=1))
            wpool = p1.enter_context(tc.tile_pool(name="wproj", bufs=1))
            qk_ps = p1.enter_context(
                tc.tile_pool(name="qk_ps", bufs=1, space="PSUM")
            )

            xT_sb = xpool.tile([P, KO, L], _F32R)
            xTr = xT[:].rearrange("(ko p) i -> p ko i", p=P)
            for mo in range(KO):
                nc.sync.dma_start(xT_sb[:, mo, :], xTr[:, mo, :])

            wq_sb = wpool.tile([P, KO, DH], _F32R)
            wk_sb = wpool.tile([P, KO, DH], _F32R)
            wv_sb = wpool.tile([P, KO, DVL], _F32R)
            nc.sync.dma_start(wq_sb[:], wq[:].rearrange("(ko p) d -> p ko d", p=P))
            nc.sync.dma_start(wk_sb[:], wk[:].rearrange("(ko p) d -> p ko d", p=P))
            nc.sync.dma_start(wv_sb[:], wv[:].rearrange("(ko p) d -> p ko d", p=P))

            # Q^T and K^T: [DH=128, L], accumulated over 12 m-chunks
            qt_ps = [qk_ps.tile([P, 512], _F32, tag=f"qt{c}", name=f"qt{c}") for c in range(3)]
            kt_ps = [qk_ps.tile([P, 512], _F32, tag=f"kt{c}", name=f"kt{c}") for c in range(3)]
            for mo in range(KO):
                for c in range(3):
                    sl = slice(c * 512, (c + 1) * 512)
                    nc.tensor.matmul(
                        qt_ps[c][:], wq_sb[:, mo, :], xT_sb[:, mo, sl],
                        start=(mo == 0), stop=(mo == KO - 1),
                    )
                    nc.tensor.matmul(
                        kt_ps[c][:], wk_sb[:, mo, :], xT_sb[:, mo, sl],
                        start=(mo == 0), stop=(mo == KO - 1),
                    )
            for c in range(3):
                sl = slice(c * 512, (c + 1) * 512)
                nc.scalar.activation(
                    QwT[:, sl], qt_ps[c][:],
                    mybir.ActivationFunctionType.Identity,
                    bias=bwT[:], scale=SCALE,
                )
                nc.scalar.activation(
                    QrT[:, sl], qt_ps[c][:],
                    mybir.ActivationFunctionType.Identity,
                    bias=brT[:], scale=SCALE,
                )
                nc.any.tensor_copy(KT[:, sl], kt_ps[c][:])

            # V: [L, DVL] natural layout, per 128-row tile
            with tc.tile_pool(name="v_ps", bufs=2, space="PSUM") as v_ps_pool:
                for jo in range(KO):
                    v_ps = v_ps_pool.tile([P, DVL], _F32)
                    for mo in range(KO):
                        nc.tensor.matmul(
                            v_ps[:],
                            xT_sb[:, mo, jo * P : (jo + 1) * P],
                            wv_sb[:, mo, :],
                            start=(mo == 0), stop=(mo == KO - 1),
                        )
                    for h in range(HL):
                        nc.any.tensor_copy(
                            Vg[:, jo, h, :DV], v_ps[:, h * DV : (h + 1) * DV]
                        )

        # ---------------- phase 1b: rKT = Wrk^T @ peT ----------------
        with ExitStack() as p15:
            pepool = p15.enter_context(tc.tile_pool(name="pe", bufs=1))
            rps = p15.enter_context(tc.tile_pool(name="r_ps", bufs=2, space="PSUM"))
            pe0 = pepool.tile([P, R + 1], _F32R)
            pe1 = pepool.tile([PF - P, R + 1], _F32R)
            wrk0 = pepool.tile([P, DH], _F32R)
            wrk1 = pepool.tile([PF - P, DH], _F32R)
            nc.sync.dma_start(pe0[:, :R], peT[:P, :])
            nc.sync.dma_start(pe1[:, :R], peT[P:, :])
            zcol = pepool.tile([P, 1], _F32)
            nc.vector.memset(zcol[:], 0.0)
            nc.any.tensor_copy(pe0[:, R : R + 1], zcol[:])
            nc.any.tensor_copy(pe1[:, R : R + 1], zcol[: PF - P])
            nc.sync.dma_start(wrk0[:], wrk[:P, :])
            nc.sync.dma_start(wrk1[:], wrk[P:, :])
            for rc in range(6):
                lo = rc * 512
                ps = rps.tile([P, 512], _F32)
                nc.tensor.matmul(
                    ps[:], wrk0[:], pe0[:, lo : lo + 512], start=True, stop=False
                )
                nc.tensor.matmul(
                    ps[:], wrk1[:], pe1[:, lo : lo + 512], start=False, stop=True
                )
                nc.any.tensor_copy(rKT[:, lo : lo + 512], ps[:])

        # ---------------- phase 2: attention ----------------
        # Per (head, group-of-4 query tiles):
        #   per query tile: rel-window matmuls -> diagonal-shift DMA;
        #   content matmuls + DVE add; row-max; in-place exp with accum_out
        #   (softmax denominator for free); in-place 1/sum normalize;
        #   PE transposes -> wT4 [j-part, 12 blocks, 512 i] (f32r).
        #   then attn^T = V-chunk.T @ wT4 with N=512 f32r matmuls,
        #   accumulated over the 12 j-blocks, evicted straight into ATT.
        with ExitStack() as p2:
            relwin_pool = p2.enter_context(tc.tile_pool(name="relwin", bufs=3))
            relsh_pool = p2.enter_context(tc.tile_pool(name="relsh", bufs=3))
            s_pool = p2.enter_context(tc.tile_pool(name="sS", bufs=4))
            wt4_pool = p2.enter_context(tc.tile_pool(name="wt4", bufs=2))
            small_pool = p2.enter_context(tc.tile_pool(name="small", bufs=6))
            rel_ps = p2.enter_context(tc.tile_pool(name="rel_ps", bufs=2, space="PSUM"))
            s_ps = p2.enter_context(tc.tile_pool(name="s_ps", bufs=2, space="PSUM"))
            t_ps = p2.enter_context(tc.tile_pool(name="t_ps", bufs=2, space="PSUM"))
            a_ps = p2.enter_context(tc.tile_pool(name="a_ps", bufs=1, space="PSUM"))

            for g in range(3):
                for h in range(HL):
                    hsl = slice(h * DQK, (h + 1) * DQK)
                    wT4 = wt4_pool.tile([P, NQ, 4 * P], _F32R)
                    for it in range(4):
                        iq = g * 4 + it
                        i0 = iq * P
                        isl = slice(i0, i0 + P)
                        c0 = (L - P) - i0  # rel-window start in rel coords

                        # rel window [P, WREL] (padded width, col WREL unused)
                        relwin = relwin_pool.tile([P, WRELP], _F32)
                        for rc in range(4):
                            lo = rc * 512
                            w = min(512, WRELP - lo)
                            ps = rel_ps.tile([P, 512], _F32)
                            nc.tensor.matmul(
                                ps[:, :w], QrT[hsl, isl],
                                rKT[hsl, c0 + lo : c0 + lo + w],
                                start=True, stop=True,
                            )
                            nc.any.tensor_copy(relwin[:, lo : lo + w], ps[:, :w])

                        # diagonal shift: relsh[p, j] = relwin[p, (P-1) - p + j]
                        relsh = relsh_pool.tile([P, L], _F32)
                        nc.sync.dma_start(
                            relsh[:], _diag_ap(relwin[:], WRELP, L, P - 1)
                        )

                        # content + relsh, then row-max
                        S = s_pool.tile([P, L], _F32)
                        for jc in range(3):
                            sl = slice(jc * 512, (jc + 1) * 512)
                            ps = s_ps.tile([P, 512], _F32)
                            nc.tensor.matmul(
                                ps[:], QwT[hsl, isl], KT[hsl, sl],
                                start=True, stop=True,
                            )
                            nc.vector.tensor_tensor(
                                S[:, sl], ps[:], relsh[:, sl], mybir.AluOpType.add
                            )
                        m = small_pool.tile([P, 1], _F32, tag="m")
                        nc.vector.reduce_max(m[:], S[:], axis=mybir.AxisListType.X)
                        negm = small_pool.tile([P, 1], _F32, tag="negm")
                        nc.vector.tensor_scalar_mul(negm[:], m[:], -1.0)

                        # in-place exp(S - rowmax), denominator via accum_out,
                        # then in-place normalize: S becomes softmax weights
                        su = small_pool.tile([P, 1], _F32, tag="su")
                        nc.scalar.activation(
                            S[:], S[:], mybir.ActivationFunctionType.Exp,
                            bias=negm[:], scale=1.0, accum_out=su[:],
                        )
                        rec = small_pool.tile([P, 1], _F32, tag="rec")
                        nc.vector.reciprocal(rec[:], su[:])
                        nc.vector.tensor_scalar_mul(S[:], S[:], rec[:])

                        # transpose w -> wT4[:, jb, it*128:(it+1)*128]
                        for gg in range(3):
                            tp = t_ps.tile([P, 512], _F32, tag="tp")
                            for k in range(4):
                                jb = gg * 4 + k
                                nc.tensor.transpose(
                                    tp[:, k * P : (k + 1) * P],
                                    S[:, jb * P : (jb + 1) * P],
                                    ident[:],
                                )
                            nc.any.tensor_copy(
                                wT4[:, gg * 4 : (gg + 1) * 4, it * P : (it + 1) * P],
                                tp[:],
                            )

                    # attn^T for this (h, g): lhsT = V chunks, rhs = wT4
                    # ATT row layout (= ATT_PERM on the host):
                    #   chunk0       = h0 v[0:128)   <- psA(h0)
                    #   chunk1       = h1 v[0:128)   <- psA(h1)
                    #   chunk2[64:]  = h0 v[128:192) <- psB(h0)[64:128]
                    #   chunk2[0:64] = h1 v[128:192) <- psB(h1)[0:64]
                    gsl = slice(g * 512, (g + 1) * 512)
                    psA = a_ps.tile([P, 4 * P], _F32, tag="A", name="psA")
                    if h == 0:
                        psB = a_ps.tile([P, 4 * P], _F32, tag="B", name="psB")
                        psBv = psB[:]
                        lhsB = lambda jb: Vg[:, jb, 0, 64:192]
                    else:
                        psB = a_ps.tile([P, 4 * P], _F32, tag="B", name="psB")
                        psBv = psB[:64, :]
                        lhsB = lambda jb: Vg[:, jb, 1, 128:192]
                    for jb in range(NQ):
                        nc.tensor.matmul(
                            psA[:], Vg[:, jb, h, 0:128], wT4[:, jb, :],
                            start=(jb == 0), stop=(jb == NQ - 1),
                        )
                        nc.tensor.matmul(
                            psBv, lhsB(jb), wT4[:, jb, :],
                            start=(jb == 0), stop=(jb == NQ - 1),
                        )
                    nc.any.tensor_copy(ATT[:, h, gsl], psA[:])
                    if h == 0:
                        nc.any.tensor_copy(ATT[64:128, 2, gsl], psB[64:128, :])
                    else:
                        nc.any.tensor_copy(ATT[0:64, 2, gsl], psB[0:64, :])


        # ---------------- phase 3: output projection ----------------
        with ExitStack() as p3:
            wopool = p3.enter_context(tc.tile_pool(name="wop", bufs=1))
            o_ps = p3.enter_context(tc.tile_pool(name="o_ps", bufs=2, space="PSUM"))
            obuf = p3.enter_context(tc.tile_pool(name="obuf", bufs=3))
            wo_sb = wopool.tile([P, 3, DM], _F32R)
            nc.sync.dma_start(wo_sb[:], wo[:].rearrange("(c p) n -> p c n", p=P))
            for iq in range(NQ):
                isl = slice(iq * P, (iq + 1) * P)
                # c outer / nck inner keeps each ATT block loaded in the PE
                # across 3 matmuls (1 LDWEIGHTS per block instead of 3)
                pss = [
                    o_ps.tile([P, 512], _F32, tag=f"o{nck}", name=f"o{nck}")
                    for nck in range(3)
                ]
                for c in range(3):
                    for nck in range(3):
                        nsl = slice(nck * 512, (nck + 1) * 512)
                        nc.tensor.matmul(
                            pss[nck][:], ATT[:, c, isl], wo_sb[:, c, nsl],
                            start=(c == 0), stop=(c == 2),
                        )
                for nck in range(3):
                    nsl = slice(nck * 512, (nck + 1) * 512)
                    ob = obuf.tile([P, 512], _F32)
                    nc.any.tensor_copy(ob[:], pss[nck][:])
                    nc.sync.dma_start(y[isl, nsl], ob[:])

    nc.compile()
    return nc


# ------------------------------------------------------------------ runner
_CACHE = {}


def _get_nc():
    if "nc" not in _CACHE:
        _CACHE["nc"] = build_kernel()
        _CACHE["peT"] = np.ascontiguousarray(_pos_features_np().T)
    return _CACHE["nc"], _CACHE["peT"]


def make_in_maps(x, Wq, Wk, Wv, Wrk, r_w_bias, r_r_bias, Wo, peT):
    in_maps = []
    for c in range(N_CORES):
        b, hp = divmod(c, 4)
        h0 = 2 * hp
        qsl = slice(h0 * DQK, h0 * DQK + DH)
        vsl = slice(h0 * DV, h0 * DV + DVL)
        in_maps.append(
            {
                "xT": np.ascontiguousarray(x[b].T),
                "wq": np.ascontiguousarray(Wq[:, qsl]),
                "wk": np.ascontiguousarray(Wk[:, qsl]),
                "wv": np.ascontiguousarray(Wv[:, vsl]),
                "wrk": np.ascontiguousarray(Wrk[:, qsl]),
                "peT": peT,
                "bw": np.ascontiguousarray(
                    r_w_bias[0, h0 : h0 + HL, 0, :].reshape(DH, 1)
                ),
                "br": np.ascontiguousarray(
                    r_r_bias[0, h0 : h0 + HL, 0, :].reshape(DH, 1)
                ),
                "wo": np.ascontiguousarray(Wo[vsl, :][ATT_PERM, :]),
            }
        )
    return in_maps


def kernel(x, Wq, Wk, Wv, Wrk, r_w_bias, r_r_bias, Wo, bo, **run_kwargs):
    x = np.asarray(x, np.float32)
    Wq = np.asarray(Wq, np.float32)
    Wk = np.asarray(Wk, np.float32)
    Wv = np.asarray(Wv, np.float32)
    Wrk = np.asarray(Wrk, np.float32)
    r_w_bias = np.asarray(r_w_bias, np.float32)
    r_r_bias = np.asarray(r_r_bias, np.float32)
    Wo = np.asarray(Wo, np.float32)
    bo = np.asarray(bo, np.float32)

    nc, peT = _get_nc()
    in_maps = make_in_maps(x, Wq, Wk, Wv, Wrk, r_w_bias, r_r_bias, Wo, peT)
    res = bass_utils.run_bass_kernel_spmd(
        nc, in_maps, core_ids=list(range(N_CORES)), **run_kwargs
    )
    out = np.zeros((B, L, DM), np.float32)
    for c in range(N_CORES):
        out[c // 4] += res.results[c]["y"]
    out += bo[None, None, :]
    if run_kwargs:
        _CACHE["last_results"] = res
    return out
